# revision 12
# baseline (speedup 1.0000x reference)
"""Bidirectional tanh-RNN (B=32, S=512, I=H=1024) on 8 Trainium2 NeuronCores.

Sharding: 2 direction groups x 4 cores (cores 0-3 fwd, 4-7 bwd; host
reverses time for bwd). Batch split 4 ways -> BL=8 sequences per core.

Per-core kernel v3 (bf16 recurrence, PE column-tiled):
  - Recurrence h @ W_hh.T as 32 matmuls/step: 8 k-tiles x 4 PE column
    groups (tile_position=(0,32g)), N=256 each.  On HW the 4 column
    groups stream concurrently (~3.4x measured vs the serial cost
    model), so the per-step MM phase is ~1.0us.
  - PSUM: 3 step-parity tiles [128,256]; has_written bits preset by
    prologue dummy groups; per-step xp is pre-copied into the parity
    tile by the POOL engine (off the ACT critical path), matmuls
    accumulate on top (start=False).
  - tanh on ACT -> Hbuf bf16 [128, 4, 256] (4-step buffer); DVE 32x32
    stream-transpose -> T tile for the next step's stationaries.
  - Output: one DMA per 4 steps of the full [128, 4*256] Hbuf slab
    (includes 96 garbage partitions; host extracts rows 32g+b).
  - xp = x @ W_ih.T + bias precomputed in 16-step chunks (f32r, N=512
    matmuls through 2 PSUM banks), DMA'd psum->DRAM directly, then
    strip-relayout DRAM->SBUF X bufs; all proj DMA issue on Pool queue.
"""

import numpy as np
from contextlib import ExitStack

import concourse.bass as bass
import concourse.mybir as mybir
import concourse.tile as tile
from concourse import bacc

F32 = mybir.dt.float32
F32R = mybir.dt.float32r
BF16 = mybir.dt.bfloat16

B, S, I, H = 32, 512, 1024, 1024
NCORES = 8
BL = 8          # local batch per core
KT = 8          # 128-row contraction tiles over I/H
CH = 16         # steps per projection chunk (M-tile of 128 = 16*8 rows)
NG = 4          # PE column groups
CW = 256        # h columns per group
HB = 8          # steps buffered per output DMA


def _emit_body(ctx: ExitStack, tc: tile.TileContext, xT, w, u, bias, out, steps):
    nc = tc.nc
    n_chunks = steps // CH
    assert steps % CH == 0
    assert steps % HB == 0

    const = ctx.enter_context(tc.tile_pool(name="const", bufs=1))
    xpool = ctx.enter_context(tc.tile_pool(name="xc", bufs=2))
    ppool = ctx.enter_context(tc.tile_pool(name="proj", bufs=2, space="PSUM"))
    rpool = ctx.enter_context(tc.tile_pool(name="rec", bufs=1, space="PSUM"))
    pspool = ctx.enter_context(tc.tile_pool(name="pjs", bufs=2))
    hpool = ctx.enter_context(tc.tile_pool(name="h", bufs=2))
    tpool = ctx.enter_context(tc.tile_pool(name="t", bufs=4))
    dpool = ctx.enter_context(tc.tile_pool(name="dram", bufs=1, space="DRAM"))

    # --- constants / resident weights ---
    w_sb = const.tile([128, KT, H], F32R)
    nc.gpsimd.dma_start(w_sb[:], w.rearrange("(k p) n -> p k n", p=128))
    u_sb = const.tile([128, KT, H], BF16)
    nc.gpsimd.dma_start(u_sb[:], u.rearrange("p (k n) -> p k n", k=KT))
    bias_sb = const.tile([1, H], F32R)
    nc.gpsimd.dma_start(bias_sb[:], bias[:])
    ones1_f = const.tile([1, 128], F32)
    nc.gpsimd.memset(ones1_f[:], 1.0)
    ones1 = const.tile([1, 128], F32R)
    nc.vector.tensor_copy(ones1[:], ones1_f[:])

    # strip-layout xp staging tiles (persistent; garbage rows zeroed once)
    X_bufs = [const.tile([128, CH, CW], F32, name=f"Xb{i}") for i in range(2)]
    for Xb in X_bufs:
        nc.gpsimd.memset(Xb[:], 0.0)

    xT_r = xT.rearrange("(k p) m -> p k m", p=128)
    xp_dram = dpool.tile([steps * BL, H], F32)
    out_r = out.rearrange("(g q b) (j c) -> g q b j c", g=NG, b=BL, c=CW)

    from collections import deque
    proj_pending = deque()

    def queue_proj(c):
        """Queue chunk c's projection matmuls as bubble-filler thunks."""
        xc = xpool.tile([128, KT, CH * BL], F32R, tag="xc")
        nc.gpsimd.dma_start(xc[:], xT_r[:, :, c * CH * BL:(c + 1) * CH * BL])
        state = {}

        def mk(k):
            def run():
                if k == -1:
                    for b2 in range(2):
                        p = ppool.tile([128, 512], F32, tag=f"pj{b2}",
                                       name=f"pj{b2}")
                        state[b2] = p
                        nc.tensor.matmul(p[:], lhsT=ones1[:],
                                         rhs=bias_sb[:, 512 * b2:512 * (b2 + 1)],
                                         start=True, stop=False)
                    return
                for b2 in range(2):
                    nbs = slice(512 * b2, 512 * (b2 + 1))
                    p = state[b2]
                    nc.tensor.matmul(p[:], lhsT=xc[:, k, :],
                                     rhs=w_sb[:, k, nbs],
                                     start=False, stop=(k == KT - 1))
                if k == KT - 1:
                    # psum -> SBUF (Pool), -> DRAM, then strip-relayout back:
                    # X[32g+b, s, c] <- xp_dram[128c + 8s + b, 256g + c]
                    xs_sb = pspool.tile([128, H], F32, tag="pjs")
                    # bank 0 copy on DVE, bank 1 on ACT: splits the spike
                    # across the two chain-critical queues.
                    nc.vector.tensor_copy(xs_sb[:, 0:512], state[0][:])
                    nc.scalar.activation(xs_sb[:, 512:1024], state[1][:],
                                         mybir.ActivationFunctionType.Copy)
                    xd = xp_dram[128 * c:128 * (c + 1), :]
                    nc.gpsimd.dma_start(xd, xs_sb[:])
                    Xb = X_bufs[c % 2]
                    for g in range(NG):
                        src = xd.rearrange(
                            "(s b) (g c) -> g b s c", s=CH, b=BL, g=NG)[g]
                        nc.gpsimd.dma_start(
                            Xb[32 * g:32 * g + BL, :, :], src)
            return run

        for k in range(-1, KT):
            proj_pending.append(mk(k))

    def drain_proj(n):
        for _ in range(min(n, len(proj_pending))):
            proj_pending.popleft()()

    # --- prologue ---
    queue_proj(0)
    drain_proj(99)

    # 3 persistent recurrence PSUM tiles (step parities). A closed dummy
    # matmul group sets the has_written bits once; they persist, so the
    # per-step start=False matmuls accumulate onto the Pool-copied xp.
    rec = []
    for par in range(3):
        r = rpool.tile([128, CW], F32, tag=f"rec{par}", name=f"rec{par}")
        nc.tensor.matmul(r[:], lhsT=ones1[:],
                         rhs=bias_sb[0:1, 0:CW], start=True, stop=True)
        rec.append(r)

    def emit_xp_copy(t):
        """DVE-copy step t's xp strips into its parity psum tile (GPSIMD
        cannot access PSUM on HW)."""
        Xb = X_bufs[(t // CH) % 2]
        nc.vector.tensor_copy(rec[t % 3][:], Xb[:, t % CH, :])

    emit_xp_copy(0)

    T_cur = None
    H_cur = None
    for t in range(steps):
        c, j = divmod(t, CH)
        if j == 0 and c + 1 < n_chunks:
            queue_proj(c + 1)

        if t + 1 < steps:
            emit_xp_copy(t + 1)

        r = rec[t % 3]
        if t > 0:
            # 32 matmuls: k-tiles outer x column-groups inner; the groups
            # stream concurrently in distinct 32-col PE strips.  The k 0-3
            # stationaries come from T half 0, k 4-7 from half 1, so the
            # first 16 matmuls only wait on the first transpose slice.
            for k in range(KT):
                lhsT = T_cur[(k >= 4)][:, 32 * (k % 4):32 * (k % 4) + BL]
                for g in range(NG):
                    nc.tensor.matmul(
                        r[32 * g:32 * g + BL, :],
                        lhsT=lhsT,
                        rhs=u_sb[:, k, CW * g:CW * (g + 1)],
                        start=False, stop=False, skip_group_check=True,
                        tile_position=(0, 32 * g))

        drain_proj(1)

        # tanh + transpose pipelined in 2 column halves:
        # tanh h0 -> transpose h0 (enables next step's k 0-3) while tanh h1
        # runs; transpose h1 enables k 4-7.
        if t % HB == 0:
            H_cur = hpool.tile([128, HB, CW], BF16, tag="h")
        h_t = H_cur[:, t % HB, :]
        T_next = [tpool.tile([128, 128], BF16, tag=f"t{i}", name=f"T{i}")
                  for i in range(2)]
        for i in range(2):
            half = slice(128 * i, 128 * (i + 1))
            nc.scalar.activation(h_t[:, half], r[:, half],
                                 mybir.ActivationFunctionType.Tanh)
            if t + 1 < steps:
                nc.vector.transpose(T_next[i][:], h_t[:, half])
        T_cur = T_next

        if t % HB == HB - 1:
            # output DMA per HB steps: one per column-group quadrant
            for g in range(NG):
                nc.sync.dma_start(out_r[g, t // HB],
                                  H_cur[32 * g:32 * g + BL, :, :])

        drain_proj(1)


def build_nc(steps=S, enable_asserts=False):
    nc = bacc.Bacc("TRN2", target_bir_lowering=False, debug=False,
                   enable_asserts=enable_asserts)
    xT = nc.dram_tensor("xT", [I, steps * BL], F32R, kind="ExternalInput").ap()
    w = nc.dram_tensor("w", [I, H], F32R, kind="ExternalInput").ap()
    u = nc.dram_tensor("u", [128, KT * H], BF16, kind="ExternalInput").ap()
    bias = nc.dram_tensor("bias", [1, H], F32R, kind="ExternalInput").ap()
    out = nc.dram_tensor("out", [NG * (steps // HB) * BL, HB * CW], BF16,
                         kind="ExternalOutput").ap()
    with tile.TileContext(nc) as tc:
        with ExitStack() as ctx:
            _emit_body(ctx, tc, xT, w, u, bias, out, steps)
    nc.compile()
    return nc


def round_f32r(a):
    """Round fp32 to the FP32R format (11 mantissa bits, RNE, low 12 bits 0)."""
    u = np.ascontiguousarray(a, dtype=np.float32).view(np.uint32)
    u = u + np.uint32(0x7FF) + ((u >> np.uint32(12)) & np.uint32(1))
    u &= np.uint32(0xFFFFF000)
    return u.view(np.float32)


def _bf16(a):
    return np.ascontiguousarray(a).astype(mybir.dt.np(mybir.dt.bfloat16))


def make_in_maps(x, W_ih_f, W_hh_f, b_ih_f, b_hh_f, W_ih_b, W_hh_b, b_ih_b, b_hh_b,
                 steps=S):
    """Build the 8 per-core input dicts. Cores 0-3 fwd, 4-7 bwd."""
    x = np.ascontiguousarray(np.asarray(x, dtype=np.float32)[:, :steps])
    sets = {}
    for d, (Wih, Whh, bi, bh) in (
            ("f", (W_ih_f, W_hh_f, b_ih_f, b_hh_f)),
            ("b", (W_ih_b, W_hh_b, b_ih_b, b_hh_b))):
        u_host = np.ascontiguousarray(np.asarray(Whh).T.astype(np.float32))
        # u_dram[p, k, n] = u_host[256*(p//32) + 32*k + p%32, n]
        u_perm = u_host.reshape(NG, KT, 32, H).transpose(0, 2, 1, 3)
        u_perm = np.ascontiguousarray(u_perm.reshape(128, KT * H))
        sets[d] = (
            round_f32r(np.ascontiguousarray(np.asarray(Wih).T.astype(np.float32))),
            _bf16(u_perm),
            round_f32r((np.asarray(bi) + np.asarray(bh)).astype(np.float32)[None, :]),
        )
    in_maps = []
    for core in range(NCORES):
        d = "f" if core < 4 else "b"
        g = core % 4
        wmat, umat, bsum = sets[d]
        xs = x[BL * g:BL * (g + 1)]
        if d == "b":
            xs = xs[:, ::-1]
        # xT[i, s*BL + b] = xs[b, s, i]
        xT = np.ascontiguousarray(xs.transpose(2, 1, 0).reshape(I, steps * BL))
        in_maps.append({
            "xT": round_f32r(xT),
            "w": wmat,
            "u": umat,
            "bias": bsum,
        })
    return in_maps


def assemble(results, steps=S):
    """results: 8 dicts with 'out' [128, steps*256] bf16 -> [B, steps, 2H]."""
    full = np.empty((B, steps, 2 * H), dtype=np.float32)
    for core in range(NCORES):
        o = np.asarray(results[core]["out"]).astype(np.float32)
        # o[gq, q, b, j, c] = h[b, q*HB + j, 256*gq + c]
        o = o.reshape(NG, steps // HB, BL, HB, CW)
        g = core % 4
        # -> [BL, steps, NG, CW]
        h = o.transpose(2, 1, 3, 0, 4).reshape(BL, steps, H)
        if core < 4:
            full[BL * g:BL * (g + 1), :, :H] = h
        else:
            full[BL * g:BL * (g + 1), :, H:] = h[:, ::-1]
    return full


def kernel(x, W_ih_f, W_hh_f, b_ih_f, b_hh_f, W_ih_b, W_hh_b, b_ih_b, b_hh_b):
    from concourse.bass_utils import run_bass_kernel_spmd
    nc = build_nc(S)
    in_maps = make_in_maps(x, W_ih_f, W_hh_f, b_ih_f, b_hh_f,
                           W_ih_b, W_hh_b, b_ih_b, b_hh_b)
    res = run_bass_kernel_spmd(nc, in_maps, list(range(NCORES))).results
    return assemble(res)
